# revision 12
# baseline (speedup 1.0000x reference)
"""IterNorm (iterative whitening normalization) Trainium2 Bass kernel.

Full-input contract: kernel(X, weight, bias) -> X_hat, shapes hardcoded for
X (64, 256, 56, 56) f32.  Data-parallel over batch N across 8 NeuronCores:
each core computes per-group raw moments (sum x, sum x x^T) over its batch
shard, an AllReduce combines them, the (g,d,d) Newton iteration for
Sigma^{-1/2} is replicated on every core, and each core applies the
whitening matrix to its shard.

Math notes:
  Sigma = eps I + (1/m) xc xc^T is computed from raw moments:
      Sigma = S2/m - mean mean^T + eps I
  All Newton iterates P_t are polynomials in the symmetric Sigma_N, hence
  symmetric; matmul lhsT arguments therefore never need transposing.
  The two 64-channel groups inside one 128-channel block are processed as a
  single block-diagonal 128x128 Newton chain.
  weight/bias and the mean subtraction are folded into the applied affine:
      out = A x + c,  A = diag(w * sqrt(rTr)) P,  c = b - A mean
"""
import os
import numpy as np

import concourse.bass as bass
import concourse.mybir as mybir
from concourse import tile, bacc
from concourse.bass_utils import run_bass_kernel_spmd

F32 = mybir.dt.float32
F32R = mybir.dt.float32r
BF16 = mybir.dt.bfloat16
ALU = mybir.AluOpType
ACTF = mybir.ActivationFunctionType

N_CORES = 8
N_PER = 8            # batch images per core
C = 256
HW = 3136            # 56*56
CB = 2               # 128-channel blocks
M_TOT = 64 * HW      # global sample count per channel row
EPS = 1e-5
T_NEWTON = 10
AF = 448             # apply free-dim chunk (HW = 7*448)
NAP = HW // AF
TK = 128             # transpose chunk
NFULL = HW // TK     # 24 full chunks ...
REM = HW - NFULL * TK  # ... + 64 remainder
N_UNITS = N_PER * CB  # (n, cb) units per core
N_CACHED = 15        # units cached in SBUF; last unit streamed


def _build(single_core=False):
    nc = bacc.Bacc("TRN2", target_bir_lowering=False, debug=False,
                   num_devices=N_CORES)
    x = nc.dram_tensor("x", [N_PER, C, HW], F32, kind="ExternalInput").ap()
    w = nc.dram_tensor("w", [C], F32, kind="ExternalInput").ap()
    b = nc.dram_tensor("b", [C], F32, kind="ExternalInput").ap()
    out = nc.dram_tensor("out", [N_PER, C, HW], F32, kind="ExternalOutput").ap()

    i128_d = nc.inline_tensor(np.eye(128, dtype=np.float32), name="ci128").ap()
    epsi_d = nc.inline_tensor(np.eye(128, dtype=np.float32) * EPS,
                              name="cepsi").ap()
    maskbd = np.zeros((128, 128), dtype=np.float32)
    maskbd[:64, :64] = 1.0
    maskbd[64:, 64:] = 1.0
    mask_d = nc.inline_tensor(maskbd, name="cmask").ap()
    indm = np.zeros((128, 2), dtype=np.float32)
    indm[:64, 0] = 1.0
    indm[64:, 1] = 1.0
    ind_d = nc.inline_tensor(indm, name="cind").ap()
    indt_d = nc.inline_tensor(np.ascontiguousarray(indm.T), name="cindt").ap()

    with tile.TileContext(nc) as tc:
        import contextlib
        es = contextlib.ExitStack()
        with es:
            consts = es.enter_context(tc.tile_pool(name="consts", bufs=1))
            cache = es.enter_context(tc.tile_pool(name="cache", bufs=N_CACHED))
            persist = es.enter_context(tc.tile_pool(name="persist", bufs=2))
            gram_es = contextlib.ExitStack()
            gram_ps = gram_es.enter_context(
                tc.tile_pool(name="gram", bufs=2, space="PSUM"))
            dram = es.enter_context(
                tc.tile_pool(name="dram", bufs=2, space="DRAM"))

            # ---- constants into SBUF ----
            i128 = consts.tile([128, 128], F32, tag="i128")
            nc.sync.dma_start(i128[:], i128_d[:])
            epsi = consts.tile([128, 128], F32, tag="epsi")
            nc.sync.dma_start(epsi[:], epsi_d[:])
            mask = consts.tile([128, 128], F32, tag="mask")
            nc.sync.dma_start(mask[:], mask_d[:])
            ind = consts.tile([128, 2], F32, tag="ind")
            nc.sync.dma_start(ind[:], ind_d[:])
            indt = consts.tile([2, 128], F32, tag="indt")
            nc.sync.dma_start(indt[:], indt_d[:])
            ones_b = consts.tile([128, 1], BF16, tag="ones_b")
            nc.vector.memset(ones_b[:], 1.0)
            w_sb = consts.tile([128, 2], F32, tag="w_sb")
            nc.sync.dma_start(w_sb[:], w.rearrange("(cb c) -> c cb", cb=2))
            b_sb = consts.tile([128, 2], F32, tag="b_sb")
            nc.sync.dma_start(b_sb[:], b.rearrange("(cb c) -> c cb", cb=2))

            # ---- per-cb gram accumulators: [c, e] block gram | col 128 sums
            gps = [gram_ps.tile([128, 129], F32, tag="gps", name=f"gps{i}")
                   for i in range(CB)]
            gram_started = [False, False]

            # =================== phase 1: load + stats ===================
            xc_tiles = []
            with tc.tile_pool(name="xt", bufs=3) as xt_pool, \
                 tc.tile_pool(name="tp", bufs=4, space="PSUM") as tp_pool, \
                 tc.tile_pool(name="scratch", bufs=1) as scratch:
                for u in range(N_UNITS):
                    n, cb = divmod(u, CB)
                    if u < N_CACHED:
                        xc_t = cache.tile([128, HW], F32, tag="cache")
                    else:
                        xc_t = scratch.tile([128, HW], F32, tag="scratch")
                    nc.sync.dma_start(xc_t[:], x[n, cb * 128:(cb + 1) * 128, :])
                    xc_tiles.append(xc_t)

                pend = []

                def flush_one():
                    xt_f, ws_f, cb_f = pend.pop(0)
                    for off, wk_f in ws_f:
                        st = not gram_started[cb_f]
                        gram_started[cb_f] = True
                        # lhsT: [m, c] slice; rhs includes the ones column so
                        # one matmul accumulates both the gram and channel sums
                        nc.tensor.matmul(gps[cb_f][:],
                                         xt_f[:wk_f, off:off + 128],
                                         xt_f[:wk_f, off:off + 129],
                                         start=st, stop=False,
                                         skip_group_check=True)

                copy_flip = 0
                for u in range(N_UNITS):
                    n, cb = divmod(u, CB)
                    xc_t = xc_tiles[u]
                    # groups of 4 transpose chunks share one PSUM bank so
                    # the PSUM->SBUF copy is one [128, 512] op
                    for g in range(7):
                        ks = [k for k in range(4 * g, min(4 * g + 4, NFULL + 1))]
                        if not ks:
                            continue
                        tp = tp_pool.tile([128, 512], F32, tag="tp")
                        ws = []
                        tot = 0
                        for i, k in enumerate(ks):
                            wk = TK if k < NFULL else REM
                            src = xc_t[:, k * TK:k * TK + wk]
                            nc.tensor.transpose(tp[:wk, i * 128:i * 128 + 128],
                                                src, i128[:])
                            ws.append((i * 129, wk))
                            tot = i * 128 + 128
                        nsl = len(ks)
                        xt = xt_pool.tile([128, 4 * 129], BF16, tag="xt")
                        xtv = xt[:].rearrange("p (s c) -> p s c", c=129)
                        nc.vector.memset(xtv[:, 0:nsl, 128:129], 1.0)
                        dst = xtv[:, 0:nsl, 0:128]
                        if copy_flip % 5 < 3:
                            nc.vector.tensor_copy(dst, tp[:, 0:tot])
                        else:
                            nc.scalar.copy(dst, tp[:, 0:tot])
                        copy_flip += 1
                        pend.append((xt, ws, cb))
                        if len(pend) > 1:
                            flush_one()
                while pend:
                    flush_one()

            # =================== phase 2: allreduce stats =================
            stats_sb = persist.tile([128, 2 * 129], F32, tag="stats")
            nc.vector.tensor_copy(stats_sb[:, 0:129], gps[0][:])
            nc.scalar.copy(stats_sb[:, 129:258], gps[1][:])
            gram_es.close()
            bounce_in = dram.tile([128, 2 * 129], F32, tag="bin")
            bounce_out = dram.tile([128, 2 * 129], F32, tag="bout")
            nc.sync.dma_start(bounce_in[:], stats_sb[:])
            if single_core:
                # timeline-profiling variant: collective replaced by a
                # same-size local copy (numerics differ, timing shape same)
                nc.sync.dma_start(bounce_out[:], bounce_in[:])
            else:
                nc.gpsimd.collective_compute(
                    "AllReduce", ALU.add,
                    replica_groups=[list(range(N_CORES))],
                    ins=[bounce_in.opt()], outs=[bounce_out.opt()])
            s_all = persist.tile([128, 2 * 129], F32, tag="sall")
            nc.sync.dma_start(s_all[:], bounce_out[:])

            # =================== phase 3: newton (replicated) =============
            ATs, cvrs = [], []
            inv_m = 1.0 / float(M_TOT)
            with tc.tile_pool(name="nw", bufs=1) as nw, \
                 tc.tile_pool(name="nwp", bufs=2) as nwp, \
                 tc.tile_pool(name="nwps", bufs=2, space="PSUM") as nwps:
                for cb in range(CB):
                    S = s_all[:, cb * 129:(cb + 1) * 129]
                    mean_c = nw.tile([128, 1], F32, tag=f"mean{cb}")
                    nc.scalar.mul(mean_c[:], S[:, 128:129], inv_m)
                    mr_ps = nwps.tile([1, 128], F32, tag="mps" + str(cb), name="mrps")
                    nc.tensor.transpose(mr_ps[:], mean_c[:], i128[:])
                    mean_r = nw.tile([1, 128], F32, tag="meanr" + str(cb))
                    nc.vector.tensor_copy(mean_r[:], mr_ps[:])
                    outer = nwps.tile([128, 128], F32, tag="mps" + str(cb), name="outer")
                    nc.tensor.matmul(outer[:], mean_r[:], mean_r[:],
                                     start=True, stop=True)
                    t_sb = nw.tile([128, 128], F32, tag="tsb" + str(cb))
                    nc.scalar.mul(t_sb[:], S[:, 0:128], inv_m)
                    sig = nw.tile([128, 128], F32, tag="sig" + str(cb))
                    nc.vector.tensor_sub(sig[:], t_sb[:], outer[:])
                    sig2 = nw.tile([128, 128], F32, tag="sig2" + str(cb))
                    nc.vector.tensor_add(sig2[:], sig[:], epsi[:])
                    dmat = nw.tile([128, 128], F32, tag="dmat" + str(cb))
                    nc.vector.tensor_mul(dmat[:], sig2[:], i128[:])
                    dcol = nw.tile([128, 1], F32, tag="dcol" + str(cb))
                    nc.vector.reduce_sum(dcol[:], dmat[:],
                                         axis=mybir.AxisListType.X)
                    tr_ps = nwps.tile([2, 1], F32, tag="mps" + str(cb), name="trps")
                    nc.tensor.matmul(tr_ps[:], ind[:], dcol[:],
                                     start=True, stop=True)
                    rtr = nw.tile([2, 1], F32, tag="rtr" + str(cb))
                    nc.vector.reciprocal(rtr[:], tr_ps[:])
                    v_ps = nwps.tile([128, 1], F32, tag="mps" + str(cb), name="vps")
                    nc.tensor.matmul(v_ps[:], indt[:], rtr[:],
                                     start=True, stop=True)
                    v_sb = nw.tile([128, 1], F32, tag="vsb" + str(cb))
                    nc.vector.tensor_copy(v_sb[:], v_ps[:])
                    sqv = nw.tile([128, 1], F32, tag="sqv" + str(cb))
                    nc.scalar.sqrt(sqv[:], v_sb[:])
                    sn0 = nw.tile([128, 128], F32, tag="sn0" + str(cb))
                    nc.scalar.mul(sn0[:], sig2[:], v_sb[:])
                    sn = nw.tile([128, 128], F32, tag="sn" + str(cb))
                    nc.vector.tensor_mul(sn[:], sn0[:], mask[:])

                    p_t = nwp.tile([128, 128], F32, tag="p" + str(cb))
                    nc.scalar.copy(p_t[:], i128[:])
                    for _ in range(T_NEWTON):
                        a_ps = nwps.tile([128, 128], F32, tag="ab" + str(cb), name="a_ps")
                        nc.tensor.matmul(a_ps[:], p_t[:], sn[:],
                                         start=True, stop=True)
                        b_ps = nwps.tile([128, 128], F32, tag="ab" + str(cb), name="bps")
                        nc.tensor.matmul(b_ps[:], p_t[:], p_t[:],
                                         start=True, stop=True)
                        a_sb = nw.tile([128, 128], F32, tag="asb" + str(cb))
                        nc.vector.tensor_copy(a_sb[:], a_ps[:])
                        b_sb2 = nw.tile([128, 128], F32, tag="bsb" + str(cb))
                        nc.scalar.copy(b_sb2[:], b_ps[:])
                        c_ps = nwps.tile([128, 128], F32, tag="ab" + str(cb), name="cps")
                        nc.tensor.matmul(c_ps[:], b_sb2[:], a_sb[:],
                                         start=True, stop=True)
                        t15 = nw.tile([128, 128], F32, tag="t15" + str(cb))
                        nc.scalar.mul(t15[:], p_t[:], 1.5)
                        p_t = nwp.tile([128, 128], F32, tag="p" + str(cb))
                        nc.vector.scalar_tensor_tensor(
                            p_t[:], c_ps[:], -0.5, t15[:], ALU.mult, ALU.add)

                    wmw = nw.tile([128, 1], F32, tag="wmw" + str(cb))
                    nc.vector.tensor_mul(wmw[:], sqv[:], w_sb[:, cb:cb + 1])
                    a_f = nw.tile([128, 128], F32, tag="af" + str(cb))
                    nc.scalar.mul(a_f[:], p_t[:], wmw[:])
                    at_ps = nwps.tile([128, 128], F32, tag="mps" + str(cb), name="atps")
                    nc.tensor.transpose(at_ps[:], a_f[:], i128[:])
                    at_sb = persist.tile([128, 128], F32, tag="at")
                    nc.vector.tensor_copy(at_sb[:], at_ps[:])
                    ATs.append(at_sb)
                    y_ps = nwps.tile([128, 1], F32, tag="mps" + str(cb), name="yps")
                    nc.tensor.matmul(y_ps[:], p_t[:], mean_c[:],
                                     start=True, stop=True)
                    ty = nw.tile([128, 1], F32, tag="ty" + str(cb))
                    nc.vector.tensor_mul(ty[:], wmw[:], y_ps[:])
                    cvc = nw.tile([128, 1], F32, tag="cvc" + str(cb))
                    nc.vector.tensor_sub(cvc[:], b_sb[:, cb:cb + 1], ty[:])

                    cvp = persist.tile([128, 1], F32, tag="cvp")
                    nc.vector.tensor_copy(cvp[:], cvc[:])
                    cvrs.append(cvp)

            # =================== phase 4: apply =========================
            with tc.tile_pool(name="osb", bufs=4) as osb, \
                 tc.tile_pool(name="aps", bufs=4, space="PSUM") as apsp, \
                 tc.tile_pool(name="stream", bufs=3) as stream:
                for cb in range(CB):
                  for n in range(N_PER):
                    u = n * CB + cb
                    for j in range(NAP):
                        if u < N_CACHED:
                            rhs = xc_tiles[u][:, j * AF:(j + 1) * AF]
                        else:
                            st = stream.tile([128, AF], F32, tag="st")
                            nc.sync.dma_start(
                                st[:], x[n, cb * 128:(cb + 1) * 128,
                                         j * AF:(j + 1) * AF])
                            rhs = st[:]
                        ap_t = apsp.tile([128, AF], F32, tag="aps")
                        nc.tensor.matmul(ap_t[:], ATs[cb][:], rhs,
                                         start=True, stop=True)
                        o_t = osb.tile([128, AF], F32, tag="osb")
                        if j % 2 == 0:
                            nc.vector.tensor_scalar_add(o_t[:], ap_t[:],
                                                        cvrs[cb][:])
                        else:
                            nc.scalar.activation(o_t[:], ap_t[:],
                                                 ACTF.Identity,
                                                 bias=cvrs[cb][:])
                        nc.sync.dma_start(
                            out[n, cb * 128:(cb + 1) * 128,
                                j * AF:(j + 1) * AF], o_t[:])

    nc.compile()
    return nc


_NC = None


def _get_nc():
    global _NC
    if _NC is None:
        _NC = _build()
    return _NC


def kernel_run(X, weight, bias, trace=False):
    X = np.ascontiguousarray(X.reshape(64, C, HW), dtype=np.float32)
    w = np.ascontiguousarray(weight.reshape(C), dtype=np.float32)
    b = np.ascontiguousarray(bias.reshape(C), dtype=np.float32)
    in_maps = [{"x": X[k * N_PER:(k + 1) * N_PER], "w": w, "b": b}
               for k in range(N_CORES)]
    res = run_bass_kernel_spmd(_get_nc(), in_maps, list(range(N_CORES)),
                               trace=trace)
    parts = [res.results[k]["out"] for k in range(N_CORES)]
    full = np.concatenate(parts, axis=0).reshape(64, C, 56, 56)
    return full, res


def kernel(X, weight, bias):
    full, _ = kernel_run(X, weight, bias,
                         trace=bool(os.environ.get("ITERNORM_TRACE")))
    return full


def model_exec_time_ns():
    """Per-core exec time from the Tile cost-model timeline (single-core
    build, collective replaced by same-size local copy). The slim axon
    client in this container has no NTFF profile hook, so this is the
    best available per-core kernel-time estimate."""
    from concourse.timeline_sim import TimelineSim
    nc1 = _build(single_core=True)
    return TimelineSim(nc1, trace=False).simulate()
